# revision 38
# baseline (speedup 1.0000x reference)
"""Multi-head self-attention TRN2 kernel (B=4, S=2048, E=1024, H=16).

Sharding: 8 cores, zero cross-core communication.  Core c handles
batch b = c//2 and query rows (c%2)*1024 : (c%2+1)*1024 of that batch.
Each core computes K/V projections for its full batch (duplicated once
per batch-pair), Q projection for its query half, attention for all 16
heads over its 1024 query rows, and the output projection for its rows.

Device notes:
- Host passes X[b].T with the core's query-half columns first, so the
  program is identical on every core (SPMD, data-varying only).
- All matmul operands are bf16 (host-converted); PSUM accumulates fp32.
- Scores are computed transposed ([k, q]); softmax denominators come
  from two all-ones columns appended to V (M=66 stationary), so the
  attention@V contraction needs no transposes anywhere.
- exp() needs no max-subtraction: scores ~ N(0,1) after the 1/sqrt(d)
  scale, comfortably inside fp32 exp range.
- AV is deferred one kt step behind exp, so the in-order PE queue never
  waits on a just-issued ACT instruction (every cross-engine dependency
  is a full ~1.15us cadence old); et ring is 3 deep to match.
- The denominator reciprocal uses the fast custom-DVE approximation
  (~18 bits, single op + f32r cast; custom-DVE ops only work from base
  partition 0, hence the [0:65] window).  The 1/d row hops to partition
  0 by a 2KB DMA, then gpsimd partition_broadcast (also base-0-only)
  fans it across 64 partitions — the whole normalize path stays off the
  PE (a K=1 matmul also paid ~150ns tile-config switch bubbles: only
  transitions in/out of the 64-row scores config cost extra; 128-config
  transitions pipeline at full rate).
- V chn0 kt 0-7 are produced in the startup DMA shadow interleaved with
  K0/Q0; kt 8-15 ride pair 0's ACT-paced slack as fillers.  V chn1 is
  spread over pairs 1-4.  Pair 7 (no successor projection) carries the
  t=0..6 partial output projection for the first q-half; the tail only
  adds the t=7 term.
- First scores of each qc/pair window are pre-issued before the prior
  window's epilogue AV, so the ACT queue never gaps at boundaries.
- A^T stays resident in SBUF ([128, 8, 1024] bf16): the normalize
  multiply writes straight into it and the output projection reads it
  as stationary tiles, so there is no DRAM staging round-trip.
- bk drops out of softmax exactly (constant shift per query row); the
  bv/bo terms commute through the output projection and are applied on
  the host as `out += bv @ Wo + bo` (exact: softmax rows sum to 1).
"""

import os
import sys

import numpy as np

if "/opt/trn_rl_repo" not in sys.path:
    sys.path.insert(0, "/opt/trn_rl_repo")

B, S, E, H = 4, 2048, 1024, 16
D = E // H            # 64
SQ = S // 2           # 1024 query rows per core
ET = E // 128         # 8 contraction tiles
KT = S // 128         # 16 key tiles
PAIRS = H // 2        # 8 head pairs (one 128-row e_out tile each)
N_CORES = 8

_CACHE = {"nc": None}
LAST_EXEC_NS = None
LAST_RESULTS = None

# Bumped on every kernel revision: sized into a dummy input so the HLO
# signature (and any fingerprint-keyed executable cache) changes too.
KERNEL_VERSION = 17


def _build_nc():
    import concourse.tile as tile
    from concourse import bacc, mybir
    from contextlib import ExitStack

    FP32 = mybir.dt.float32
    F32R = mybir.dt.float32r
    BF16 = mybir.dt.bfloat16
    FP8 = mybir.dt.float8e4
    DR = mybir.MatmulPerfMode.DoubleRow
    AF = mybir.ActivationFunctionType

    nc = bacc.Bacc("TRN2", target_bir_lowering=False, debug=False,
                   num_devices=N_CORES)

    xt = nc.dram_tensor("xt", [E, S], BF16, kind="ExternalInput").ap()
    wq = nc.dram_tensor("wq", [E, E], BF16, kind="ExternalInput").ap()
    wk = nc.dram_tensor("wk", [E, E], BF16, kind="ExternalInput").ap()
    wv = nc.dram_tensor("wv", [E, E], BF16, kind="ExternalInput").ap()
    wo = nc.dram_tensor("wo", [E, E], BF16, kind="ExternalInput").ap()
    bqp = nc.dram_tensor("bqp", [128, PAIRS], FP32, kind="ExternalInput").ap()
    bkp = nc.dram_tensor("bkp", [128, PAIRS], FP32, kind="ExternalInput").ap()
    vone = nc.dram_tensor("vone", [128, 64], FP32, kind="ExternalInput").ap()
    ver = nc.dram_tensor("ver", [1, KERNEL_VERSION], FP32,
                         kind="ExternalInput").ap()
    out = nc.dram_tensor("out", [SQ, E], FP32, kind="ExternalOutput").ap()

    # DRAM views with the e_in (contraction) dim split onto partitions.
    xt_t = xt.rearrange("(t p) k -> p t k", p=128)     # [128, 8, 2048]
    wq_t = wq.rearrange("(t p) m -> p t m", p=128)     # [128, 8, 1024]
    wk_t = wk.rearrange("(t p) m -> p t m", p=128)
    wv_t = wv.rearrange("(t p) m -> p t m", p=128)
    wo_t = wo.rearrange("(t p) m -> p t m", p=128)

    with tile.TileContext(nc) as tc, ExitStack() as ctx:
        aux = ctx.enter_context(tc.tile_pool(name="aux", bufs=1))
        vone_sb = aux.tile([128, 64], F32R)
        nc.sync.dma_start(vone_sb[:], vone[:].bitcast(F32R))
        bqp_sb = aux.tile([128, PAIRS], FP32)
        bkp_sb = aux.tile([128, PAIRS], FP32)
        # softmax reciprocal staging; only partition 64 is ever read.
        # 4 slots = (qc parity, head) so deferred normalizes never WAR-stall.
        rec_sb = aux.tile([65, 4, 512], F32R)
        rec0 = aux.tile([1, 4, 512], F32R)
        rec_f32 = aux.tile([65, 4, 512], FP32)
        ver_sb = aux.tile([1, KERNEL_VERSION], FP32)

        vp = ctx.enter_context(tc.tile_pool(name="vp", bufs=1))
        # V natural (k on partitions), 66 cols/head: 64 data + 2 ones.
        V = vp.tile([128, KT, H, 66], BF16)
        nc.vector.memset(V[:, :, :, 64:66], 1.0)

        # A^T, SBUF-resident: e_out rows on partitions, q free.
        atp_sb = ctx.enter_context(tc.tile_pool(name="atsb", bufs=1))
        AT = atp_sb.tile([128, ET, SQ], BF16)

        xtp = ctx.enter_context(tc.tile_pool(name="xtp", bufs=1))
        XT = xtp.tile([128, ET, S], BF16)       # X^T, e_in on partitions

        pair_ctx = ExitStack()
        kqp = pair_ctx.enter_context(tc.tile_pool(name="kqp", bufs=2))
        qqp = pair_ctx.enter_context(tc.tile_pool(name="qqp", bufs=2))
        wkq = pair_ctx.enter_context(tc.tile_pool(name="wkq", bufs=2))
        # 3-deep: exp(kt) writes one slot while the deferred AV(kt-1)
        # reads another; the third gives the ACT queue a full step of
        # slack so no engine ever waits on a just-issued producer.
        etp = pair_ctx.enter_context(tc.tile_pool(name="etp", bufs=3))
        atp = pair_ctx.enter_context(tc.tile_pool(name="atp", bufs=2))
        rbp = pair_ctx.enter_context(tc.tile_pool(name="rbp", bufs=2))
        pkq = pair_ctx.enter_context(
            tc.tile_pool(name="pkq", bufs=1, space="PSUM"))
        psc = pair_ctx.enter_context(
            tc.tile_pool(name="psc", bufs=2, space="PSUM"))
        pvbc = pair_ctx.enter_context(
            tc.tile_pool(name="pvbc", bufs=1, space="PSUM"))
        pat = pair_ctx.enter_context(
            tc.tile_pool(name="pat", bufs=1, space="PSUM"))

        def load_w_pair(j):
            wk_j = wkq.tile([128, ET, 128], BF16, tag="wk")
            nc.sync.dma_start(wk_j[:], wk_t[:, :, j * 128:(j + 1) * 128])
            wq_j = wkq.tile([128, ET, 128], BF16, tag="wq")
            nc.sync.dma_start(wq_j[:], wq_t[:, :, j * 128:(j + 1) * 128])
            return wk_j, wq_j

        def proj_chunk(w_j, dst, bias, j, ch):
            pk = pkq.tile([128, 512], FP32, tag="pkq")
            for t in range(ET):
                nc.tensor.matmul(
                    pk[:], w_j[:, t, :],
                    XT[:, t, ch * 512:(ch + 1) * 512],
                    start=(t == 0), stop=(t == ET - 1))
            with nc.allow_low_precision(reason="bf16 KQ rounding"):
                nc.vector.tensor_scalar_add(
                    dst[:, ch * 512:(ch + 1) * 512], pk[:],
                    bias[:, j:j + 1])

        def proj_pair(j, wk_j, wq_j):
            Kj = kqp.tile([128, S], BF16, tag="kt")    # K^T rows, 2 heads
            for ch in range(4):
                proj_chunk(wk_j, Kj, bkp_sb, j, ch)
            Qj = qqp.tile([128, SQ], BF16, tag="qt")   # Q^T rows, 2 heads
            for ch in range(2):
                proj_chunk(wq_j, Qj, bqp_sb, j, ch)
            return Kj, Qj

        def proj_micro(j, wk_j, wq_j):
            # One matmul (or one PSUM->SBUF add) per closure, so filler
            # work interleaves into the kt loop at sub-0.3us granularity.
            Kj = kqp.tile([128, S], BF16, tag="kt", name=f"K{j}")
            Qj = qqp.tile([128, SQ], BF16, tag="qt", name=f"Q{j}")
            ops = []
            for w_j, dst, bias, nch, pf in ((wk_j, Kj, bkp_sb, 4, "k"),
                                            (wq_j, Qj, bqp_sb, 2, "q")):
                for ch in range(nch):
                    box = {}

                    def mm(t, w_j=w_j, ch=ch, box=box, pf=pf):
                        if t == 0:
                            box["pk"] = pkq.tile(
                                [128, 512], FP32, tag="pkq",
                                name=f"p{pf}{j}{ch}")
                        nc.tensor.matmul(
                            box["pk"][:], w_j[:, t, :],
                            XT[:, t, ch * 512:(ch + 1) * 512],
                            start=(t == 0), stop=(t == ET - 1))

                    def add(dst=dst, bias=bias, ch=ch, box=box):
                        with nc.allow_low_precision(reason="bf16 KQ round"):
                            nc.vector.tensor_scalar_add(
                                dst[:, ch * 512:(ch + 1) * 512],
                                box["pk"][:], bias[:, j:j + 1])

                    ops += [lambda t=t, mm=mm: mm(t) for t in range(ET)]
                    ops.append(add)
            return Kj, Qj, ops

        def v_micro(chn, kt):
            box = {}

            def mm(t):
                if t == 0:
                    pool = psc if chn == 0 else pvbc
                    box["pv"] = pool.tile(
                        [128, 512], FP32,
                        tag="sc" if chn == 0 else "pv",
                        name=f"pv{chn}_{kt}")
                nc.tensor.matmul(
                    box["pv"][:],
                    XT[:, t, kt * 128:(kt + 1) * 128],
                    v_w[chn][:, t, :],
                    start=(t == 0), stop=(t == ET - 1))

            def cp():
                nc.vector.tensor_copy(
                    V[:, kt, chn * 8:(chn + 1) * 8, 0:64],
                    box["pv"][:].rearrange("p (h d) -> p h d", d=64))

            return [lambda t=t: mm(t) for t in range(ET)] + [cp]

        # startup: XT's first key-column chunks lead the DMA queue so the
        # K0/V0 stream can start ~9us in; weights ride just behind.
        wvp = pair_ctx.enter_context(tc.tile_pool(name="wvp", bufs=2))
        for th in range(2):
            nc.sync.dma_start(
                XT[:, th * 4:(th + 1) * 4, 0:512],
                xt_t[:, th * 4:(th + 1) * 4, 0:512])
        wk_0, wq_0 = load_w_pair(0)
        nc.sync.dma_start(bqp_sb[:], bqp[:])
        nc.sync.dma_start(bkp_sb[:], bkp[:])
        v_w = {}
        v_w[0] = wvp.tile([128, ET, 512], BF16, tag="wvh", name="wv0")
        nc.sync.dma_start(v_w[0][:], wv_t[:, :, 0:512])
        for kc in range(1, 4):
            nc.sync.dma_start(
                XT[:, :, kc * 512:(kc + 1) * 512],
                xt_t[:, :, kc * 512:(kc + 1) * 512])
        v_w[1] = wvp.tile([128, ET, 512], BF16, tag="wvh", name="wv1")
        nc.sync.dma_start(v_w[1][:], wv_t[:, :, 512:1024])
        nc.sync.dma_start(ver_sb[:], ver[:])
        # ~10us of dummy matmuls on the tiny ones tile while the XT DMA is
        # in flight: trips the HAM activity window so the real projection
        # stream starts at the full 2.4 GHz clock.
        warm = pkq.tile([64, 64], FP32, tag="pkq")
        for _ in range(36):
            nc.tensor.matmul(warm[:], vone_sb[:, :], vone_sb[:, :],
                             start=True, stop=True)

        # ---- V projection: V[k, e] = X @ Wv (no bias; host handles) ----
        def v_kt(chn, kt):
            Wv_sb = v_w[chn]
            pool = psc if chn == 0 else pvbc
            pv = pool.tile([128, 512], FP32,
                           tag="sc" if chn == 0 else "pv")
            for t in range(ET):
                nc.tensor.matmul(
                    pv[:],
                    XT[:, t, kt * 128:(kt + 1) * 128],
                    Wv_sb[:, t, :],
                    start=(t == 0), stop=(t == ET - 1))
            nc.vector.tensor_copy(
                V[:, kt, chn * 8:(chn + 1) * 8, 0:64],
                pv[:].rearrange("p (h d) -> p h d", d=64))

        # K0/Q0 interleaved with the whole V chn0 sweep: all of it rides
        # the XT/Wv DMA shadow, and pair 0's attention is then purely
        # ACT-paced instead of carrying 27us of jit V work.
        K0 = kqp.tile([128, S], BF16, tag="kt", name="K0")
        for ch in range(4):
            proj_chunk(wk_0, K0, bkp_sb, 0, ch)
            for kt in range(4 * ch, 4 * ch + 4):
                v_kt(0, kt)
        Q0 = qqp.tile([128, SQ], BF16, tag="qt", name="Q0")
        for ch in range(2):
            proj_chunk(wq_0, Q0, bqp_sb, 0, ch)

        # Deferred-normalize pipeline: each (pair, qc) drains its attn
        # banks with plain copies; the reciprocals run in the next kt
        # loop's quiet DVE window and the broadcast matmul + normalize
        # multiply are emitted mid-way through it, so neither the PE nor
        # the attn-bank recycle ever waits on the 3.3us reciprocal.
        pending = []

        def drain_qc(j, qc, attn):
            entry = [j, qc, [], False]
            for h in range(2):
                a = atp.tile([65, 512], FP32, tag=f"ats{h}")
                nc.vector.tensor_copy(a[:], attn[h][0:65, :])
                entry[2].append(a)
            pending.append(entry)

        def emit_recip_chunk():
            # fast custom-DVE reciprocal (seed + 2 inline NR passes, ~18
            # bits) into fp32 scratch, then a cast-copy to the f32r row the
            # broadcast matmul reads: ~1.2us/head vs 3.4us InstReciprocal.
            entry = pending[0]
            j, qc, ats, done = entry
            if done >= 4:
                return
            h, phase = divmod(done, 2)
            s = (qc % 2) * 2 + h
            if phase == 0:
                # custom-DVE ops require base partition 0: run over the
                # whole [0:65] tile (same cost, DVE time is free-size
                # driven); only row 64 (the denominators) is read below.
                nc.vector.reciprocal_approx_fast(
                    rec_f32[0:65, s, :], ats[h][0:65, :])
            else:
                with nc.allow_low_precision(reason="f32r denom"):
                    nc.vector.tensor_copy(rec_sb[64:65, s, :],
                                          rec_f32[64:65, s, :])
                # hop the 1/d row to partition 0: partition_broadcast is
                # only correct from base 0, and DVE ops cannot shift
                # partitions; a 2KB SBUF->SBUF DMA can.
                nc.sync.dma_start(rec0[0:1, s, :], rec_sb[64:65, s, :])
            entry[3] = done + 1

        def emit_recips():
            while pending and pending[0][3] < 4:
                emit_recip_chunk()

        def emit_normalize():
            emit_recips()
            j, qc, ats, _ = pending.pop(0)
            qsl = slice(qc * 512, (qc + 1) * 512)
            for h in range(2):
                s = (qc % 2) * 2 + h
                # partition-broadcast the 1/d row on the (otherwise idle)
                # GpSimd engine: keeps the whole normalize path off the PE
                # (the old K=1 f32r matmul also paid ~150ns config-switch
                # bubbles per call).
                bc = rbp.tile([64, 512], F32R, tag="rb")
                nc.gpsimd.partition_broadcast(bc[:], rec0[0:1, s, :])
                with nc.allow_low_precision(reason="bf16 normalize"):
                    nc.vector.tensor_mul(
                        AT[h * 64:h * 64 + 64, j, qsl],
                        ats[h][0:64, :], bc[:])

        def attention_pair(j, Kj, Qj, fillers=(), micro=()):
            # fillers: sorted (step, fn) hard-scheduled; micro: flat list
            # of sub-0.3us closures spread evenly over the 32 steps.
            fq = sorted(fillers, key=lambda sf: sf[0])
            mq = list(micro)
            n_micro = len(mq)
            for qc in range(2):
                qsl = slice(qc * 512, (qc + 1) * 512)
                attn0 = pat.tile([128, 512], FP32, tag="attn0")
                attn1 = pat.tile([128, 512], FP32, tag="attn1")
                attn = [attn0, attn1]

                def scores(kt):
                    ksl = slice(kt * 128, (kt + 1) * 128)
                    sc = psc.tile([128, 2, 512], FP32, tag="sc")
                    for h in range(2):
                        hsl = slice(h * 64, (h + 1) * 64)
                        nc.tensor.matmul(sc[:, h, :], Kj[hsl, ksl],
                                         Qj[hsl, qsl],
                                         start=True, stop=True)
                    return sc

                def av(akt, aet):
                    for h in range(2):
                        nc.tensor.matmul(
                            attn[h][0:66, :],
                            V[:, akt, 2 * j + h, :],
                            aet[:, h, :],
                            start=(akt == 0), stop=(akt == KT - 1))

                sc_cur = scores(0)
                av_prev = None
                for kt in range(KT):
                    step = qc * KT + kt
                    if step in (1, 3, 5, 7, 17, 19, 21, 23) and pending:
                        emit_recip_chunk()
                    if step in (12, 28) and pending and pending[0][3] >= 4:
                        emit_normalize()
                    # AV deferred one step: by emission time its exp is a
                    # full cadence old, so the PE queue head never waits on
                    # a just-issued ACT instruction.
                    if av_prev is not None:
                        av(*av_prev)
                    et = etp.tile([128, 2, 512], BF16)
                    nc.scalar.activation(et[:], sc_cur[:], AF.Exp,
                                         scale=0.125)
                    if kt + 1 < KT:
                        sc_next = scores(kt + 1)
                    while fq and fq[0][0] <= step:
                        fq.pop(0)[1]()
                    want = n_micro * (step + 1) // 32
                    while len(mq) > n_micro - want:
                        mq.pop(0)()
                    av_prev = (kt, et)
                    if kt + 1 < KT:
                        sc_cur = sc_next
                av(*av_prev)
                drain_qc(j, qc, attn)
            for _, f in fq:
                f()
            for f in mq:
                f()

        # Wo can land any time before the output projection.
        wop = pair_ctx.enter_context(tc.tile_pool(name="wop", bufs=1))
        Wo_sb = []
        for chh in range(2):
            w = wop.tile([128, ET, 512], BF16, tag=f"wo{chh}")
            nc.sync.dma_start(w[:], wo_t[:, :, chh * 512:(chh + 1) * 512])
            Wo_sb.append(w)

        KQ = {0: (K0, Q0)}
        for j in range(PAIRS):
            fillers = []
            micro = []
            if j == 0:
                # V chn0 kt 8-15 ride pair 0's ACT-paced slack; tile kt is
                # only needed by the deferred AV at step kt+1.
                fillers += [(kt - 7, lambda kt=kt: v_kt(0, kt))
                            for kt in range(8, KT)]
            elif j <= 3 and j >= 1:
                # V chn1 (heads 8-15): pairs 1-3 carry 4 tiles each;
                # pair 4 takes the last 4 just-in-time.
                for kt in range(4 * (j - 1), 4 * j):
                    micro += v_micro(1, kt)
            elif j == 4:
                fillers += [(11 + i, lambda kt=kt: v_kt(1, kt))
                            for i, kt in enumerate(range(12, KT))]
            if j + 1 < PAIRS:
                wk_n, wq_n = load_w_pair(j + 1)
                Kn, Qn, ops = proj_micro(j + 1, wk_n, wq_n)
                KQ[j + 1] = (Kn, Qn)
                micro += ops
            attention_pair(j, *KQ[j], fillers=fillers, micro=micro)
        if pending:
            emit_recips()   # last qc's reciprocals overlap early O-proj

        # ---- output projection: out[q, e] = A @ Wo (no bias; host) ----
        # qt-major: qt 0-3 only need the earlier-flushed q-half, letting
        # the final pending normalize overlap with real PE work.
        with tc.tile_pool(name="osp", bufs=4) as osp:
            for qt in range(8):
                if qt == 4:
                    while pending:
                        emit_normalize()
                for ch in range(2):
                    po = psc.tile([128, 512], FP32, tag="sc")
                    for t in range(ET):
                        nc.tensor.matmul(
                            po[:], AT[:, t, qt * 128:(qt + 1) * 128],
                            Wo_sb[ch][:, t, :],
                            start=(t == 0), stop=(t == ET - 1))
                    o_sb = osp.tile([128, 512], FP32)
                    nc.vector.tensor_copy(o_sb[:], po[:])
                    nc.sync.dma_start(
                        out[qt * 128:(qt + 1) * 128,
                            ch * 512:(ch + 1) * 512], o_sb[:])
        pair_ctx.close()

    nc.compile()
    return nc


def _host_inputs(inputs, Wq, bq, Wk, bk, Wv, bv, Wo, bo):
    import ml_dtypes

    f = np.float32
    bf = ml_dtypes.bfloat16
    wq16 = np.ascontiguousarray(np.asarray(Wq, f).astype(bf))
    wk16 = np.ascontiguousarray(np.asarray(Wk, f).astype(bf))
    wv16 = np.ascontiguousarray(np.asarray(Wv, f).astype(bf))
    wo16 = np.ascontiguousarray(np.asarray(Wo, f).astype(bf))
    bqp = np.ascontiguousarray(np.asarray(bq, f).reshape(PAIRS, 128).T)
    bkp = np.ascontiguousarray(np.asarray(bk, f).reshape(PAIRS, 128).T)
    vone = np.ones((128, 64), f)

    in_maps = []
    for c in range(N_CORES):
        b, half = divmod(c, 2)
        X = np.asarray(inputs[b], f)              # [S, E]
        qlo = half * SQ
        xt = np.empty((E, S), f)
        xt[:, :SQ] = X[qlo:qlo + SQ].T            # query half first
        xt[:, SQ:] = X[SQ - qlo:S - qlo].T        # the other half
        in_maps.append({
            "xt": np.ascontiguousarray(xt.astype(bf)),
            "wq": wq16, "wk": wk16, "wv": wv16, "wo": wo16,
            "bqp": bqp, "bkp": bkp, "vone": vone,
            "ver": np.zeros((1, KERNEL_VERSION), f),
        })
    return in_maps


def kernel(inputs, Wq, bq, Wk, bk, Wv, bv, Wo, bo):
    global LAST_EXEC_NS, LAST_RESULTS
    from concourse.bass_utils import run_bass_kernel_spmd

    if _CACHE["nc"] is None:
        _CACHE["nc"] = _build_nc()
    nc = _CACHE["nc"]

    in_maps = _host_inputs(inputs, Wq, bq, Wk, bk, Wv, bv, Wo, bo)
    tmpdir = os.environ.get("KERNEL_TMPDIR")
    if tmpdir:
        os.makedirs(tmpdir, exist_ok=True)
    res = run_bass_kernel_spmd(
        nc, in_maps, core_ids=list(range(N_CORES)),
        tmpdir=tmpdir,
        trace=bool(os.environ.get("KERNEL_TRACE")))
    LAST_EXEC_NS = res.exec_time_ns
    LAST_RESULTS = res

    # bv/bo commute through the output projection: softmax rows sum to 1,
    # so attn(v + bv) = attn(v) + bv and (A + bv) @ Wo + bo = A@Wo + fix.
    fix = (np.asarray(bv, np.float32) @ np.asarray(Wo, np.float32)
           + np.asarray(bo, np.float32))
    out = np.empty((B, S, E), np.float32)
    for c in range(N_CORES):
        b, half = divmod(c, 2)
        out[b, half * SQ:(half + 1) * SQ, :] = res.results[c]["out"] + fix
    return out



# revision 39
# speedup vs baseline: 1.1860x; 1.1860x over previous
"""Multi-head self-attention TRN2 kernel (B=4, S=2048, E=1024, H=16).

Sharding: 8 cores, zero cross-core communication.  Core c handles
batch b = c//2 and query rows (c%2)*1024 : (c%2+1)*1024 of that batch.
Each core computes K/V projections for its full batch (duplicated once
per batch-pair), Q projection for its query half, attention for all 16
heads over its 1024 query rows, and the output projection for its rows.

Device notes:
- Host passes X[b].T with the core's query-half columns first, so the
  program is identical on every core (SPMD, data-varying only).
- All matmul operands are bf16 (host-converted); PSUM accumulates fp32.
- Scores are computed transposed ([k, q]); softmax denominators come
  from two all-ones columns appended to V (M=66 stationary), so the
  attention@V contraction needs no transposes anywhere.
- exp() needs no max-subtraction: scores ~ N(0,1) after the 1/sqrt(d)
  scale, comfortably inside fp32 exp range.
- AV is deferred one kt step behind exp, so the in-order PE queue never
  waits on a just-issued ACT instruction (every cross-engine dependency
  is a full ~1.15us cadence old); et ring is 3 deep to match.
- The denominator reciprocal uses the fast custom-DVE approximation
  (~18 bits, single op + f32r cast; custom-DVE ops only work from base
  partition 0, hence the [0:65] window).  The 1/d row hops to partition
  0 by a 2KB DMA, then gpsimd partition_broadcast (also base-0-only)
  fans it across 64 partitions — the whole normalize path stays off the
  PE (a K=1 matmul also paid ~150ns tile-config switch bubbles: only
  transitions in/out of the 64-row scores config cost extra; 128-config
  transitions pipeline at full rate).
- V chn0 kt 0-7 are produced in the startup DMA shadow interleaved with
  K0/Q0; kt 8-15 ride pair 0's ACT-paced slack as fillers.  V chn1 is
  spread over pairs 1-4.  Pair 7 (no successor projection) carries the
  t=0..6 partial output projection for the first q-half; the tail only
  adds the t=7 term.
- First scores of each qc/pair window are pre-issued before the prior
  window's epilogue AV, so the ACT queue never gaps at boundaries.
- A^T stays resident in SBUF ([128, 8, 1024] bf16): the normalize
  multiply writes straight into it and the output projection reads it
  as stationary tiles, so there is no DRAM staging round-trip.
- bk drops out of softmax exactly (constant shift per query row); the
  bv/bo terms commute through the output projection and are applied on
  the host as `out += bv @ Wo + bo` (exact: softmax rows sum to 1).
"""

import os
import sys

import numpy as np

if "/opt/trn_rl_repo" not in sys.path:
    sys.path.insert(0, "/opt/trn_rl_repo")

B, S, E, H = 4, 2048, 1024, 16
D = E // H            # 64
SQ = S // 2           # 1024 query rows per core
ET = E // 128         # 8 contraction tiles
KT = S // 128         # 16 key tiles
PAIRS = H // 2        # 8 head pairs (one 128-row e_out tile each)
N_CORES = 8

_CACHE = {"nc": None}
LAST_EXEC_NS = None
LAST_RESULTS = None

# Bumped on every kernel revision: sized into a dummy input so the HLO
# signature (and any fingerprint-keyed executable cache) changes too.
KERNEL_VERSION = 17


def _build_nc():
    import concourse.tile as tile
    from concourse import bacc, mybir
    from contextlib import ExitStack

    FP32 = mybir.dt.float32
    F32R = mybir.dt.float32r
    BF16 = mybir.dt.bfloat16
    FP8 = mybir.dt.float8e4
    DR = mybir.MatmulPerfMode.DoubleRow
    AF = mybir.ActivationFunctionType

    nc = bacc.Bacc("TRN2", target_bir_lowering=False, debug=False,
                   num_devices=N_CORES)

    xt = nc.dram_tensor("xt", [E, S], BF16, kind="ExternalInput").ap()
    wq = nc.dram_tensor("wq", [E, E], BF16, kind="ExternalInput").ap()
    wk = nc.dram_tensor("wk", [E, E], BF16, kind="ExternalInput").ap()
    wv = nc.dram_tensor("wv", [E, E], BF16, kind="ExternalInput").ap()
    wo = nc.dram_tensor("wo", [E, E], BF16, kind="ExternalInput").ap()
    bqp = nc.dram_tensor("bqp", [128, PAIRS], FP32, kind="ExternalInput").ap()
    bkp = nc.dram_tensor("bkp", [128, PAIRS], FP32, kind="ExternalInput").ap()
    vone = nc.dram_tensor("vone", [128, 64], FP32, kind="ExternalInput").ap()
    ver = nc.dram_tensor("ver", [1, KERNEL_VERSION], FP32,
                         kind="ExternalInput").ap()
    out = nc.dram_tensor("out", [SQ, E], FP32, kind="ExternalOutput").ap()

    # DRAM views with the e_in (contraction) dim split onto partitions.
    xt_t = xt.rearrange("(t p) k -> p t k", p=128)     # [128, 8, 2048]
    wq_t = wq.rearrange("(t p) m -> p t m", p=128)     # [128, 8, 1024]
    wk_t = wk.rearrange("(t p) m -> p t m", p=128)
    wv_t = wv.rearrange("(t p) m -> p t m", p=128)
    wo_t = wo.rearrange("(t p) m -> p t m", p=128)

    with tile.TileContext(nc) as tc, ExitStack() as ctx:
        aux = ctx.enter_context(tc.tile_pool(name="aux", bufs=1))
        vone_sb = aux.tile([128, 64], F32R)
        nc.sync.dma_start(vone_sb[:], vone[:].bitcast(F32R))
        bqp_sb = aux.tile([128, PAIRS], FP32)
        bkp_sb = aux.tile([128, PAIRS], FP32)
        # softmax reciprocal staging; only partition 64 is ever read.
        # 4 slots = (qc parity, head) so deferred normalizes never WAR-stall.
        rec_sb = aux.tile([65, 4, 512], F32R)
        rec0 = aux.tile([1, 4, 512], F32R)
        rec_f32 = aux.tile([65, 4, 512], FP32)
        ver_sb = aux.tile([1, KERNEL_VERSION], FP32)

        vp = ctx.enter_context(tc.tile_pool(name="vp", bufs=1))
        # V natural (k on partitions), 66 cols/head: 64 data + 2 ones.
        V = vp.tile([128, KT, H, 66], BF16)
        nc.vector.memset(V[:, :, :, 64:66], 1.0)

        # A^T, SBUF-resident: e_out rows on partitions, q free.
        atp_sb = ctx.enter_context(tc.tile_pool(name="atsb", bufs=1))
        AT = atp_sb.tile([128, ET, SQ], BF16)

        xtp = ctx.enter_context(tc.tile_pool(name="xtp", bufs=1))
        XT = xtp.tile([128, ET, S], BF16)       # X^T, e_in on partitions

        pair_ctx = ExitStack()
        kqp = pair_ctx.enter_context(tc.tile_pool(name="kqp", bufs=2))
        qqp = pair_ctx.enter_context(tc.tile_pool(name="qqp", bufs=2))
        wkq = pair_ctx.enter_context(tc.tile_pool(name="wkq", bufs=2))
        # 3-deep: exp(kt) writes one slot while the deferred AV(kt-1)
        # reads another; the third gives the ACT queue a full step of
        # slack so no engine ever waits on a just-issued producer.
        etp = pair_ctx.enter_context(tc.tile_pool(name="etp", bufs=3))
        atp = pair_ctx.enter_context(tc.tile_pool(name="atp", bufs=2))
        rbp = pair_ctx.enter_context(tc.tile_pool(name="rbp", bufs=2))
        pkq = pair_ctx.enter_context(
            tc.tile_pool(name="pkq", bufs=1, space="PSUM"))
        psc = pair_ctx.enter_context(
            tc.tile_pool(name="psc", bufs=2, space="PSUM"))
        pvbc = pair_ctx.enter_context(
            tc.tile_pool(name="pvbc", bufs=1, space="PSUM"))
        pat = pair_ctx.enter_context(
            tc.tile_pool(name="pat", bufs=1, space="PSUM"))

        def load_w_pair(j):
            wk_j = wkq.tile([128, ET, 128], BF16, tag="wk")
            nc.sync.dma_start(wk_j[:], wk_t[:, :, j * 128:(j + 1) * 128])
            wq_j = wkq.tile([128, ET, 128], BF16, tag="wq")
            nc.sync.dma_start(wq_j[:], wq_t[:, :, j * 128:(j + 1) * 128])
            return wk_j, wq_j

        def proj_chunk(w_j, dst, bias, j, ch):
            pk = pkq.tile([128, 512], FP32, tag="pkq")
            for t in range(ET):
                nc.tensor.matmul(
                    pk[:], w_j[:, t, :],
                    XT[:, t, ch * 512:(ch + 1) * 512],
                    start=(t == 0), stop=(t == ET - 1))
            with nc.allow_low_precision(reason="bf16 KQ rounding"):
                nc.vector.tensor_scalar_add(
                    dst[:, ch * 512:(ch + 1) * 512], pk[:],
                    bias[:, j:j + 1])

        def proj_pair(j, wk_j, wq_j):
            Kj = kqp.tile([128, S], BF16, tag="kt")    # K^T rows, 2 heads
            for ch in range(4):
                proj_chunk(wk_j, Kj, bkp_sb, j, ch)
            Qj = qqp.tile([128, SQ], BF16, tag="qt")   # Q^T rows, 2 heads
            for ch in range(2):
                proj_chunk(wq_j, Qj, bqp_sb, j, ch)
            return Kj, Qj

        def proj_micro(j, wk_j, wq_j):
            # One matmul (or one PSUM->SBUF add) per closure, so filler
            # work interleaves into the kt loop at sub-0.3us granularity.
            Kj = kqp.tile([128, S], BF16, tag="kt", name=f"K{j}")
            Qj = qqp.tile([128, SQ], BF16, tag="qt", name=f"Q{j}")
            ops = []
            for w_j, dst, bias, nch, pf in ((wk_j, Kj, bkp_sb, 4, "k"),
                                            (wq_j, Qj, bqp_sb, 2, "q")):
                for ch in range(nch):
                    box = {}

                    def mm(t, w_j=w_j, ch=ch, box=box, pf=pf):
                        if t == 0:
                            box["pk"] = pkq.tile(
                                [128, 512], FP32, tag="pkq",
                                name=f"p{pf}{j}{ch}")
                        nc.tensor.matmul(
                            box["pk"][:], w_j[:, t, :],
                            XT[:, t, ch * 512:(ch + 1) * 512],
                            start=(t == 0), stop=(t == ET - 1))

                    def add(dst=dst, bias=bias, ch=ch, box=box):
                        with nc.allow_low_precision(reason="bf16 KQ round"):
                            nc.vector.tensor_scalar_add(
                                dst[:, ch * 512:(ch + 1) * 512],
                                box["pk"][:], bias[:, j:j + 1])

                    ops += [lambda t=t, mm=mm: mm(t) for t in range(ET)]
                    ops.append(add)
            return Kj, Qj, ops

        def v_micro(chn, kt):
            box = {}

            def mm(t):
                if t == 0:
                    pool = psc if chn == 0 else pvbc
                    box["pv"] = pool.tile(
                        [128, 512], FP32,
                        tag="sc" if chn == 0 else "pv",
                        name=f"pv{chn}_{kt}")
                nc.tensor.matmul(
                    box["pv"][:],
                    XT[:, t, kt * 128:(kt + 1) * 128],
                    v_w[chn][:, t, :],
                    start=(t == 0), stop=(t == ET - 1))

            def cp():
                nc.vector.tensor_copy(
                    V[:, kt, chn * 8:(chn + 1) * 8, 0:64],
                    box["pv"][:].rearrange("p (h d) -> p h d", d=64))

            return [lambda t=t: mm(t) for t in range(ET)] + [cp]

        # startup: XT's first key-column chunks lead the DMA queue so the
        # K0/V0 stream can start ~9us in; weights ride just behind.
        wvp = pair_ctx.enter_context(tc.tile_pool(name="wvp", bufs=2))
        for th in range(2):
            nc.sync.dma_start(
                XT[:, th * 4:(th + 1) * 4, 0:512],
                xt_t[:, th * 4:(th + 1) * 4, 0:512])
        wk_0, wq_0 = load_w_pair(0)
        nc.sync.dma_start(bqp_sb[:], bqp[:])
        nc.sync.dma_start(bkp_sb[:], bkp[:])
        v_w = {}
        v_w[0] = wvp.tile([128, ET, 512], BF16, tag="wvh", name="wv0")
        nc.sync.dma_start(v_w[0][:], wv_t[:, :, 0:512])
        for kc in range(1, 4):
            nc.sync.dma_start(
                XT[:, :, kc * 512:(kc + 1) * 512],
                xt_t[:, :, kc * 512:(kc + 1) * 512])
        v_w[1] = wvp.tile([128, ET, 512], BF16, tag="wvh", name="wv1")
        nc.sync.dma_start(v_w[1][:], wv_t[:, :, 512:1024])
        nc.sync.dma_start(ver_sb[:], ver[:])
        # ~10us of dummy matmuls on the tiny ones tile while the XT DMA is
        # in flight: trips the HAM activity window so the real projection
        # stream starts at the full 2.4 GHz clock.
        warm = pkq.tile([64, 64], FP32, tag="pkq")
        for _ in range(36):
            nc.tensor.matmul(warm[:], vone_sb[:, :], vone_sb[:, :],
                             start=True, stop=True)

        # ---- V projection: V[k, e] = X @ Wv (no bias; host handles) ----
        def v_kt(chn, kt):
            Wv_sb = v_w[chn]
            pool = psc if chn == 0 else pvbc
            pv = pool.tile([128, 512], FP32,
                           tag="sc" if chn == 0 else "pv")
            for t in range(ET):
                nc.tensor.matmul(
                    pv[:],
                    XT[:, t, kt * 128:(kt + 1) * 128],
                    Wv_sb[:, t, :],
                    start=(t == 0), stop=(t == ET - 1))
            nc.vector.tensor_copy(
                V[:, kt, chn * 8:(chn + 1) * 8, 0:64],
                pv[:].rearrange("p (h d) -> p h d", d=64))

        # K0/Q0 interleaved with the whole V chn0 sweep: all of it rides
        # the XT/Wv DMA shadow, and pair 0's attention is then purely
        # ACT-paced instead of carrying 27us of jit V work.
        K0 = kqp.tile([128, S], BF16, tag="kt", name="K0")
        for ch in range(4):
            proj_chunk(wk_0, K0, bkp_sb, 0, ch)
            for kt in range(4 * ch, min(4 * ch + 4, 8)):
                v_kt(0, kt)
        Q0 = qqp.tile([128, SQ], BF16, tag="qt", name="Q0")
        for ch in range(2):
            proj_chunk(wq_0, Q0, bqp_sb, 0, ch)

        # Deferred-normalize pipeline: each (pair, qc) drains its attn
        # banks with plain copies; the reciprocals run in the next kt
        # loop's quiet DVE window and the broadcast matmul + normalize
        # multiply are emitted mid-way through it, so neither the PE nor
        # the attn-bank recycle ever waits on the 3.3us reciprocal.
        pending = []

        def drain_qc(j, qc, attn):
            entry = [j, qc, [], False]
            for h in range(2):
                a = atp.tile([65, 512], FP32, tag=f"ats{h}")
                nc.vector.tensor_copy(a[:], attn[h][0:65, :])
                entry[2].append(a)
            pending.append(entry)

        def emit_recip_chunk():
            # fast custom-DVE reciprocal (seed + 2 inline NR passes, ~18
            # bits) into fp32 scratch, then a cast-copy to the f32r row the
            # broadcast matmul reads: ~1.2us/head vs 3.4us InstReciprocal.
            entry = pending[0]
            j, qc, ats, done = entry
            if done >= 4:
                return
            h, phase = divmod(done, 2)
            s = (qc % 2) * 2 + h
            if phase == 0:
                # custom-DVE ops require base partition 0: run over the
                # whole [0:65] tile (same cost, DVE time is free-size
                # driven); only row 64 (the denominators) is read below.
                nc.vector.reciprocal_approx_fast(
                    rec_f32[0:65, s, :], ats[h][0:65, :])
            else:
                with nc.allow_low_precision(reason="f32r denom"):
                    nc.vector.tensor_copy(rec_sb[64:65, s, :],
                                          rec_f32[64:65, s, :])
                # hop the 1/d row to partition 0: partition_broadcast is
                # only correct from base 0, and DVE ops cannot shift
                # partitions; a 2KB SBUF->SBUF DMA can.
                nc.sync.dma_start(rec0[0:1, s, :], rec_sb[64:65, s, :])
            entry[3] = done + 1

        def emit_recips():
            while pending and pending[0][3] < 4:
                emit_recip_chunk()

        def emit_normalize():
            emit_recips()
            j, qc, ats, _ = pending.pop(0)
            qsl = slice(qc * 512, (qc + 1) * 512)
            for h in range(2):
                s = (qc % 2) * 2 + h
                # partition-broadcast the 1/d row on the (otherwise idle)
                # GpSimd engine: keeps the whole normalize path off the PE
                # (the old K=1 f32r matmul also paid ~150ns config-switch
                # bubbles per call).
                bc = rbp.tile([64, 512], F32R, tag="rb")
                nc.gpsimd.partition_broadcast(bc[:], rec0[0:1, s, :])
                with nc.allow_low_precision(reason="bf16 normalize"):
                    nc.vector.tensor_mul(
                        AT[h * 64:h * 64 + 64, j, qsl],
                        ats[h][0:64, :], bc[:])

        def attention_pair(j, Kj, Qj, fillers=(), micro=()):
            # fillers: sorted (step, fn) hard-scheduled; micro: flat list
            # of sub-0.3us closures spread evenly over the 32 steps.
            fq = sorted(fillers, key=lambda sf: sf[0])
            mq = list(micro)
            n_micro = len(mq)
            for qc in range(2):
                qsl = slice(qc * 512, (qc + 1) * 512)
                attn0 = pat.tile([128, 512], FP32, tag="attn0")
                attn1 = pat.tile([128, 512], FP32, tag="attn1")
                attn = [attn0, attn1]

                def scores(kt):
                    ksl = slice(kt * 128, (kt + 1) * 128)
                    sc = psc.tile([128, 2, 512], FP32, tag="sc")
                    for h in range(2):
                        hsl = slice(h * 64, (h + 1) * 64)
                        nc.tensor.matmul(sc[:, h, :], Kj[hsl, ksl],
                                         Qj[hsl, qsl],
                                         start=True, stop=True)
                    return sc

                def av(akt, aet):
                    for h in range(2):
                        nc.tensor.matmul(
                            attn[h][0:66, :],
                            V[:, akt, 2 * j + h, :],
                            aet[:, h, :],
                            start=(akt == 0), stop=(akt == KT - 1))

                sc_cur = scores(0)
                av_prev = None
                for kt in range(KT):
                    step = qc * KT + kt
                    if step in (1, 3, 5, 7, 17, 19, 21, 23) and pending:
                        emit_recip_chunk()
                    if step in (12, 28) and pending and pending[0][3] >= 4:
                        emit_normalize()
                    # AV deferred one step: by emission time its exp is a
                    # full cadence old, so the PE queue head never waits on
                    # a just-issued ACT instruction.
                    if av_prev is not None:
                        av(*av_prev)
                    et = etp.tile([128, 2, 512], BF16)
                    nc.scalar.activation(et[:], sc_cur[:], AF.Exp,
                                         scale=0.125)
                    if kt + 1 < KT:
                        sc_next = scores(kt + 1)
                    while fq and fq[0][0] <= step:
                        fq.pop(0)[1]()
                    want = n_micro * (step + 1) // 32
                    while len(mq) > n_micro - want:
                        mq.pop(0)()
                    av_prev = (kt, et)
                    if kt + 1 < KT:
                        sc_cur = sc_next
                av(*av_prev)
                drain_qc(j, qc, attn)
            for _, f in fq:
                f()
            for f in mq:
                f()

        # Wo can land any time before the output projection.
        wop = pair_ctx.enter_context(tc.tile_pool(name="wop", bufs=1))
        Wo_sb = []
        for chh in range(2):
            w = wop.tile([128, ET, 512], BF16, tag=f"wo{chh}")
            nc.sync.dma_start(w[:], wo_t[:, :, chh * 512:(chh + 1) * 512])
            Wo_sb.append(w)

        KQ = {0: (K0, Q0)}
        for j in range(PAIRS):
            fillers = []
            micro = []
            if j == 0:
                # V chn0 kt 8-15 ride pair 0's ACT-paced slack; tile kt is
                # only needed by the deferred AV at step kt+1.
                fillers += [(kt - 7, lambda kt=kt: v_kt(0, kt))
                            for kt in range(8, KT)]
            elif j <= 3 and j >= 1:
                # V chn1 (heads 8-15): pairs 1-3 carry 4 tiles each;
                # pair 4 takes the last 4 just-in-time.
                for kt in range(4 * (j - 1), 4 * j):
                    micro += v_micro(1, kt)
            elif j == 4:
                fillers += [(11 + i, lambda kt=kt: v_kt(1, kt))
                            for i, kt in enumerate(range(12, KT))]
            if j + 1 < PAIRS:
                wk_n, wq_n = load_w_pair(j + 1)
                Kn, Qn, ops = proj_micro(j + 1, wk_n, wq_n)
                KQ[j + 1] = (Kn, Qn)
                micro += ops
            attention_pair(j, *KQ[j], fillers=fillers, micro=micro)
        if pending:
            emit_recips()   # last qc's reciprocals overlap early O-proj

        # ---- output projection: out[q, e] = A @ Wo (no bias; host) ----
        # qt-major: qt 0-3 only need the earlier-flushed q-half, letting
        # the final pending normalize overlap with real PE work.
        with tc.tile_pool(name="osp", bufs=4) as osp:
            for qt in range(8):
                if qt == 4:
                    while pending:
                        emit_normalize()
                for ch in range(2):
                    po = psc.tile([128, 512], FP32, tag="sc")
                    for t in range(ET):
                        nc.tensor.matmul(
                            po[:], AT[:, t, qt * 128:(qt + 1) * 128],
                            Wo_sb[ch][:, t, :],
                            start=(t == 0), stop=(t == ET - 1))
                    o_sb = osp.tile([128, 512], FP32)
                    nc.vector.tensor_copy(o_sb[:], po[:])
                    nc.sync.dma_start(
                        out[qt * 128:(qt + 1) * 128,
                            ch * 512:(ch + 1) * 512], o_sb[:])
        pair_ctx.close()

    nc.compile()
    return nc


def _host_inputs(inputs, Wq, bq, Wk, bk, Wv, bv, Wo, bo):
    import ml_dtypes

    f = np.float32
    bf = ml_dtypes.bfloat16
    wq16 = np.ascontiguousarray(np.asarray(Wq, f).astype(bf))
    wk16 = np.ascontiguousarray(np.asarray(Wk, f).astype(bf))
    wv16 = np.ascontiguousarray(np.asarray(Wv, f).astype(bf))
    wo16 = np.ascontiguousarray(np.asarray(Wo, f).astype(bf))
    bqp = np.ascontiguousarray(np.asarray(bq, f).reshape(PAIRS, 128).T)
    bkp = np.ascontiguousarray(np.asarray(bk, f).reshape(PAIRS, 128).T)
    vone = np.ones((128, 64), f)

    in_maps = []
    for c in range(N_CORES):
        b, half = divmod(c, 2)
        X = np.asarray(inputs[b], f)              # [S, E]
        qlo = half * SQ
        xt = np.empty((E, S), f)
        xt[:, :SQ] = X[qlo:qlo + SQ].T            # query half first
        xt[:, SQ:] = X[SQ - qlo:S - qlo].T        # the other half
        in_maps.append({
            "xt": np.ascontiguousarray(xt.astype(bf)),
            "wq": wq16, "wk": wk16, "wv": wv16, "wo": wo16,
            "bqp": bqp, "bkp": bkp, "vone": vone,
            "ver": np.zeros((1, KERNEL_VERSION), f),
        })
    return in_maps


def kernel(inputs, Wq, bq, Wk, bk, Wv, bv, Wo, bo):
    global LAST_EXEC_NS, LAST_RESULTS
    from concourse.bass_utils import run_bass_kernel_spmd

    if _CACHE["nc"] is None:
        _CACHE["nc"] = _build_nc()
    nc = _CACHE["nc"]

    in_maps = _host_inputs(inputs, Wq, bq, Wk, bk, Wv, bv, Wo, bo)
    tmpdir = os.environ.get("KERNEL_TMPDIR")
    if tmpdir:
        os.makedirs(tmpdir, exist_ok=True)
    res = run_bass_kernel_spmd(
        nc, in_maps, core_ids=list(range(N_CORES)),
        tmpdir=tmpdir,
        trace=bool(os.environ.get("KERNEL_TRACE")))
    LAST_EXEC_NS = res.exec_time_ns
    LAST_RESULTS = res

    # bv/bo commute through the output projection: softmax rows sum to 1,
    # so attn(v + bv) = attn(v) + bv and (A + bv) @ Wo + bo = A@Wo + fix.
    fix = (np.asarray(bv, np.float32) @ np.asarray(Wo, np.float32)
           + np.asarray(bo, np.float32))
    out = np.empty((B, S, E), np.float32)
    for c in range(N_CORES):
        b, half = divmod(c, 2)
        out[b, half * SQ:(half + 1) * SQ, :] = res.results[c]["out"] + fix
    return out

